# revision 5
# baseline (speedup 1.0000x reference)
"""BehaviorMoE Trainium2 kernel (8 NeuronCores, SPMD data-parallel over sorted tokens).

Contract: kernel(**inputs) takes FULL inputs as returned by setup_inputs() and
returns the FULL [8192, 1024] float32 output.

Strategy:
  - Host: sort tokens by behavior id. Tokens with b==0 need no expert compute
    (output = x + beta); they are used as masked filler so that every core gets
    exactly 1024 tokens that share a single behavior id.  Each core receives
    the stacked weight matrix [W_sh0; W_sh1; W_sh2; W_sp[t]]^T for its behavior.
  - Device (identical SPMD program, per-core data): for each 128-token tile,
    compute gate logits (PE), softmax (DVE/ACT), transpose gates (PE),
    bias combine via gates^T @ b_all (PE), the 4 expert outputs via fp32r
    matmuls (PE, full rate at N=512), the gated combine (DVE fused
    scalar_tensor_tensor), LayerNorm + residual, and DMA out.
  - Host: scatter per-core outputs back to original token order.
"""

import os
import sys

import numpy as np

for _p in ("/opt/trn_rl_repo", "/root/.axon_site/_ro/trn_rl_repo"):
    if os.path.isdir(_p) and _p not in sys.path:
        sys.path.append(_p)

from contextlib import ExitStack

from concourse import bacc, bass, masks, mybir, tile
from concourse.bass_utils import run_bass_kernel_spmd

F32 = mybir.dt.float32
F32R = mybir.dt.float32r
AX = mybir.AxisListType
ALU = mybir.AluOpType
ACTF = mybir.ActivationFunctionType

D = 1024            # model dim
N = 8192            # tokens
NB = 4              # behaviors
NESH = 3            # shared experts
NE = 4              # experts per behavior (3 shared + 1 specific)
EPS = 1e-5
NCORES = 8
M = N // NCORES     # tokens per core
KT = D // 128       # k tiles (contraction)
IT = M // 128       # token tiles per core
FH = 512            # feature half-tile (psum bank width in f32)
NF = NE * D // FH   # 8 feature tiles of 512 over the 4 stacked experts


def _build_program(trivial_affine: bool) -> bass.Bass:
    nc = bacc.Bacc()

    xt_d = nc.declare_dram_parameter("xt", [IT, KT, 128, 128], F32R, isOutput=False)
    xtok_d = nc.declare_dram_parameter("xtok", [M, D], F32, isOutput=False)
    wt_d = nc.declare_dram_parameter("wt", [KT, NF, 128, FH], F32R, isOutput=False)
    wg_d = nc.declare_dram_parameter("wg", [128, KT * NE], F32R, isOutput=False)
    ball_d = nc.declare_dram_parameter("ball", [NE, D], F32R, isOutput=False)
    mask_d = nc.declare_dram_parameter("mask", [IT, 128, 1], F32, isOutput=False)
    if not trivial_affine:
        gam_d = nc.declare_dram_parameter("gam", [128, D], F32, isOutput=False)
        bet_d = nc.declare_dram_parameter("bet", [128, D], F32, isOutput=False)
    out_d = nc.declare_dram_parameter("out", [M, D], F32, isOutput=True)

    with tile.TileContext(nc) as tc, ExitStack() as ctx:
        const = ctx.enter_context(tc.tile_pool(name="const", bufs=1))
        wpool = ctx.enter_context(tc.tile_pool(name="w", bufs=KT * NF))
        xpool = ctx.enter_context(tc.tile_pool(name="xk", bufs=2 * KT))
        xtokp = ctx.enter_context(tc.tile_pool(name="xtok", bufs=2))
        selp = ctx.enter_context(tc.tile_pool(name="sel", bufs=8))
        lnp = ctx.enter_context(tc.tile_pool(name="ln", bufs=4))
        outp = ctx.enter_context(tc.tile_pool(name="outp", bufs=2))
        smallp = ctx.enter_context(tc.tile_pool(name="small", bufs=24))
        scrp = ctx.enter_context(tc.tile_pool(name="scr", bufs=2))
        zpool = ctx.enter_context(
            tc.tile_pool(name="z", bufs=6, space="PSUM")
        )
        pspool = ctx.enter_context(
            tc.tile_pool(name="ps", bufs=2, space="PSUM")
        )

        identity = const.tile([128, 128], F32, tag="ident")
        masks.make_identity(nc, identity[:])
        wg_sb = const.tile([128, KT * NE], F32R, tag="wg")
        nc.sync.dma_start(wg_sb[:], wg_d[:])
        ball_sb = const.tile([NE, D], F32R, tag="ball")
        nc.sync.dma_start(ball_sb[:], ball_d[:])
        if not trivial_affine:
            gam_sb = const.tile([128, D], F32, tag="gam")
            nc.sync.dma_start(gam_sb[:], gam_d[:])
            bet_sb = const.tile([128, D], F32, tag="bet")
            nc.sync.dma_start(bet_sb[:], bet_d[:])

        # resident stacked weights, f-major so early feature tiles land first
        w_sb = [[None] * NF for _ in range(KT)]
        for f in range(NF):
            for k in range(KT):
                t = wpool.tile([128, FH], F32R, tag="w")
                nc.sync.dma_start(t[:], wt_d[k, f])
                w_sb[k][f] = t

        for i in range(IT):
            xk = []
            for k in range(KT):
                t = xpool.tile([128, 128], F32R, tag="xk")
                nc.sync.dma_start(t[:], xt_d[i, k])
                xk.append(t)
            mask_t = smallp.tile([128, 1], F32, tag="mask")
            nc.sync.dma_start(mask_t[:], mask_d[i])
            xi = xtokp.tile([128, D], F32, tag="xtok")
            nc.sync.dma_start(xi[:], xtok_d[i * 128:(i + 1) * 128, :])

            # ---- gate logits: [128 tok, 4] ----
            glp = pspool.tile([128, NE], F32, tag="ps")
            for k in range(KT):
                nc.tensor.matmul(
                    glp[:],
                    xk[k][:],
                    wg_sb[:, k * NE:(k + 1) * NE],
                    start=(k == 0),
                    stop=(k == KT - 1),
                )

            # ---- first expert's z matmuls keep PE busy during softmax ----
            zp = {}
            for c in (0, 1):
                zt = zpool.tile([128, FH], F32, tag="z")
                for k in range(KT):
                    nc.tensor.matmul(
                        zt[:],
                        xk[k][:],
                        w_sb[k][0 * 2 + c][:],
                        start=(k == 0),
                        stop=(k == KT - 1),
                    )
                zp[(0, c)] = zt

            # ---- softmax over the 4 gate slots (free dim), masked ----
            negmax = smallp.tile([128, 1], F32, tag="s1")
            nc.vector.tensor_reduce(
                negmax[:], glp[:], axis=AX.X, op=ALU.max, negate=True
            )
            exps = smallp.tile([128, NE], F32, tag="s4")
            expsum = smallp.tile([128, 1], F32, tag="s1")
            nc.scalar.activation(
                exps[:], glp[:], ACTF.Exp,
                bias=negmax[:], scale=1.0, accum_out=expsum[:],
            )
            rinv = smallp.tile([128, 1], F32, tag="s1")
            nc.vector.reciprocal(rinv[:], expsum[:])
            rm = smallp.tile([128, 1], F32, tag="s1")
            nc.vector.tensor_mul(rm[:], rinv[:], mask_t[:])
            gates = smallp.tile([128, NE], F32, tag="s4")
            nc.vector.tensor_scalar_mul(gates[:], exps[:], rm[:])

            # ---- transpose gates -> [4, 128] for the bias matmul ----
            gtp = pspool.tile([NE, 128], F32, tag="ps")
            nc.tensor.transpose(gtp[:], gates[:], identity[:])
            gT = smallp.tile([NE, 128], F32R, tag="gT")
            nc.vector.tensor_copy(gT[:], gtp[:])

            # more z matmuls while bias combine is prepared
            for c in (0, 1):
                zt = zpool.tile([128, FH], F32, tag="z")
                for k in range(KT):
                    nc.tensor.matmul(
                        zt[:],
                        xk[k][:],
                        w_sb[k][1 * 2 + c][:],
                        start=(k == 0),
                        stop=(k == KT - 1),
                    )
                zp[(1, c)] = zt

            # ---- bias combine: sel0 = gates^T-weighted biases ----
            sel = {}
            for c in (0, 1):
                bps = pspool.tile([128, FH], F32, tag="ps")
                nc.tensor.matmul(
                    bps[:],
                    gT[:],
                    ball_sb[:, c * FH:(c + 1) * FH],
                    start=True,
                    stop=True,
                )
                s0 = selp.tile([128, FH], F32, tag="sel")
                nc.scalar.copy(s0[:], bps[:])
                sel[c] = s0

            # ---- gated combine over experts ----
            halfsum = {}
            for e in range(NE):
                if e >= 2:
                    for c in (0, 1):
                        zt = zpool.tile([128, FH], F32, tag="z")
                        for k in range(KT):
                            nc.tensor.matmul(
                                zt[:],
                                xk[k][:],
                                w_sb[k][e * 2 + c][:],
                                start=(k == 0),
                                stop=(k == KT - 1),
                            )
                        zp[(e, c)] = zt
                for c in (0, 1):
                    cur = selp.tile([128, FH], F32, tag="sel")
                    if e == NE - 1:
                        hs = smallp.tile([128, 1], F32, tag="s1")
                        nc.vector.scalar_tensor_tensor(
                            cur[:], zp[(e, c)][:], gates[:, e:e + 1], sel[c][:],
                            op0=ALU.mult, op1=ALU.add, accum_out=hs[:],
                        )
                        halfsum[c] = hs
                    else:
                        nc.vector.scalar_tensor_tensor(
                            cur[:], zp[(e, c)][:], gates[:, e:e + 1], sel[c][:],
                            op0=ALU.mult, op1=ALU.add,
                        )
                    sel[c] = cur

            # ---- LayerNorm stats ----
            sumv = smallp.tile([128, 1], F32, tag="s1")
            nc.vector.tensor_add(sumv[:], halfsum[0][:], halfsum[1][:])
            scr0 = scrp.tile([128, FH], F32, tag="scr")
            sq0 = smallp.tile([128, 1], F32, tag="s1")
            nc.scalar.activation(scr0[:], sel[0][:], ACTF.Square, accum_out=sq0[:])
            scr1 = scrp.tile([128, FH], F32, tag="scr")
            sq1 = smallp.tile([128, 1], F32, tag="s1")
            nc.scalar.activation(scr1[:], sel[1][:], ACTF.Square, accum_out=sq1[:])
            sq = smallp.tile([128, 1], F32, tag="s1")
            nc.vector.tensor_add(sq[:], sq0[:], sq1[:])
            s2 = smallp.tile([128, 1], F32, tag="s1")
            nc.scalar.square(s2[:], sumv[:])
            varn = smallp.tile([128, 1], F32, tag="s1")
            nc.vector.scalar_tensor_tensor(
                varn[:], s2[:], -1.0 / D, sq[:], op0=ALU.mult, op1=ALU.add
            )
            av = smallp.tile([128, 1], F32, tag="s1")
            nc.vector.tensor_scalar(
                av[:], varn[:], 1.0 / D, EPS, op0=ALU.mult, op1=ALU.add
            )
            sd = smallp.tile([128, 1], F32, tag="s1")
            nc.scalar.sqrt(sd[:], av[:])
            rstd = smallp.tile([128, 1], F32, tag="s1")
            nc.vector.reciprocal(rstd[:], sd[:])
            mbt = smallp.tile([128, 1], F32, tag="s1")
            nc.vector.tensor_mul(mbt[:], sumv[:], rstd[:])
            mb = smallp.tile([128, 1], F32, tag="s1")
            nc.vector.tensor_scalar_mul(mb[:], mbt[:], -1.0 / D)

            # ---- normalize + (affine) + residual ----
            outt = outp.tile([128, D], F32, tag="out")
            for c in (0, 1):
                cs = slice(c * FH, (c + 1) * FH)
                lnc = lnp.tile([128, FH], F32, tag="ln")
                nc.scalar.activation(
                    lnc[:], sel[c][:], ACTF.Identity, bias=mb[:], scale=rstd[:]
                )
                if trivial_affine:
                    nc.vector.tensor_add(outt[:, cs], lnc[:], xi[:, cs])
                else:
                    lng = lnp.tile([128, FH], F32, tag="ln")
                    nc.vector.tensor_mul(lng[:], lnc[:], gam_sb[:, cs])
                    lnb = lnp.tile([128, FH], F32, tag="ln")
                    nc.vector.tensor_add(lnb[:], lng[:], bet_sb[:, cs])
                    nc.vector.tensor_add(outt[:, cs], lnb[:], xi[:, cs])
            nc.sync.dma_start(out_d[i * 128:(i + 1) * 128, :], outt[:])

    nc.finalize()
    return nc


_PROGRAM_CACHE: dict = {}


def _get_program(trivial_affine: bool) -> bass.Bass:
    key = trivial_affine
    if key not in _PROGRAM_CACHE:
        _PROGRAM_CACHE[key] = _build_program(trivial_affine)
    return _PROGRAM_CACHE[key]


def _pack_tokens(b: np.ndarray):
    """Partition 8192 tokens into 8 chunks of 1024, each chunk holding tokens
    of a single behavior (1..4) plus masked b==0 filler."""
    idx0 = np.flatnonzero(b == 0)
    chunks = []
    for t in range(1, NB + 1):
        idxs = np.flatnonzero(b == t)
        for s in range(0, max(len(idxs), 1), M):
            part = idxs[s:s + M]
            if len(part) or not chunks:
                chunks.append((part, t))
    chunks = [(p, t) for (p, t) in chunks if len(p) > 0]
    if len(chunks) > NCORES:
        raise RuntimeError(
            f"token packing needs {len(chunks)} single-behavior chunks > {NCORES}"
        )
    while len(chunks) < NCORES:
        chunks.append((np.empty((0,), np.int64), 1))
    p0 = 0
    cores = []
    for part, t in chunks:
        need = M - len(part)
        fill = idx0[p0:p0 + need]
        p0 += need
        if len(fill) != need:
            raise RuntimeError("not enough b==0 filler tokens for packing")
        idx = np.concatenate([part.astype(np.int64), fill.astype(np.int64)])
        msk = np.zeros((M,), np.float32)
        msk[:len(part)] = 1.0
        cores.append((idx, msk, t))
    assert p0 == len(idx0)
    return cores


def _behavior_tensors(W_sh, b_sh, W_sp, b_sp, w_gates):
    per_t = {}
    W_sh_flat = W_sh.reshape(NESH * D, D)
    for t in range(1, NB + 1):
        Wall = np.concatenate([W_sh_flat, W_sp[t - 1:t].reshape(D, D)], axis=0)
        wT = np.ascontiguousarray(Wall.T)                      # [D, 4*D]
        wt_h = np.ascontiguousarray(
            wT.reshape(KT, 128, NF, FH).transpose(0, 2, 1, 3)
        )                                                      # [k, f, 128, FH]
        wg_h = np.ascontiguousarray(
            w_gates[t - 1].reshape(KT, 128, NE).transpose(1, 0, 2).reshape(128, KT * NE)
        )
        ball_h = np.ascontiguousarray(
            np.stack([b_sh[0], b_sh[1], b_sh[2], b_sp[t - 1]], axis=0)
        )                                                      # [4, D]
        per_t[t] = (wt_h, wg_h, ball_h)
    return per_t


def _prepare(x, b_seq, W_sh, b_sh, W_sp, b_sp, w_gates, gamma, beta):
    x = np.ascontiguousarray(np.asarray(x, dtype=np.float32))
    b = np.asarray(b_seq).astype(np.int64).ravel()
    W_sh = np.asarray(W_sh, dtype=np.float32)
    b_sh = np.asarray(b_sh, dtype=np.float32)
    W_sp = np.asarray(W_sp, dtype=np.float32)
    b_sp = np.asarray(b_sp, dtype=np.float32)
    w_gates = np.asarray(w_gates, dtype=np.float32)
    gamma = np.asarray(gamma, dtype=np.float32)
    beta = np.asarray(beta, dtype=np.float32)
    assert x.shape == (N, D) and b.shape == (N,)

    trivial = bool(np.all(gamma == 1.0) and np.all(beta == 0.0))
    cores = _pack_tokens(b)
    per_t = _behavior_tensors(W_sh, b_sh, W_sp, b_sp, w_gates)

    in_maps = []
    for idx, msk, t in cores:
        wt_h, wg_h, ball_h = per_t[t]
        xc = np.ascontiguousarray(x[idx])                      # [M, D]
        xt_h = np.ascontiguousarray(
            xc.T.reshape(KT, 128, IT, 128).transpose(2, 0, 1, 3)
        )                                                      # [i, k, 128, 128]
        m = {
            "xt": xt_h,
            "xtok": xc,
            "wt": wt_h,
            "wg": wg_h,
            "ball": ball_h,
            "mask": np.ascontiguousarray(msk.reshape(IT, 128, 1)),
        }
        if not trivial:
            m["gam"] = np.ascontiguousarray(np.broadcast_to(gamma, (128, D)))
            m["bet"] = np.ascontiguousarray(np.broadcast_to(beta, (128, D)))
        in_maps.append(m)
    return trivial, cores, in_maps


def kernel_with_results(trace: bool = False, **inputs):
    trivial, cores, in_maps = _prepare(**inputs)
    nc = _get_program(trivial)
    res = run_bass_kernel_spmd(
        nc, in_maps, list(range(NCORES)), trace=trace
    )
    out = np.empty((N, D), np.float32)
    for c, (idx, _msk, _t) in enumerate(cores):
        out[idx] = res.results[c]["out"]
    return out, res


def kernel(**inputs) -> np.ndarray:
    out, _ = kernel_with_results(trace=False, **inputs)
    return out
